# revision 5
# baseline (speedup 1.0000x reference)
"""Trainium2 Bass kernel for nn_BaseTree (decision-tree inference).

Problem: x [524288, 32] f32; perfect binary tree depth 8 (255 branch nodes,
256 leaves); out[b] = value[leaf(b)] where leaf(b) is found by descending the
tree: at node n go right iff x[b, feature[n]] > threshold[n].

This environment (axon/PJRT bass execution) runs bass instructions fully
serialized with a ~25-30us fixed overhead per instruction plus a strong
penalty for non-contiguous inner access patterns (contiguous u8 ops stream at
~0.34ns/elem; gather ops cost ~177us each) -- measured via probes.  The
kernel is therefore shaped to minimize instruction count and keep every hot
op's inner dimension contiguous:

  - Pure data parallel across 8 cores (65536 rows each); tree tables are
    baked into the compiled program as immediates.
  - Device, per 256-row-per-partition tile (2 tiles per core, ~9
    instructions each): ONE giant is_gt computes all 255 node comparisons
    per row against a plane-major threshold table (entry e = plane*32 + f
    holds the plane-th node splitting on feature f, padded with +inf).  The
    input AP broadcasts x[r, f] across planes with a stride-0 MIDDLE dim so
    the innermost dim stays contiguous.  Then 3 scalar_tensor_tensor folds
    per 8-plane group ((hi*2^k) + lo on contiguous 128/64/32-entry blocks)
    pack the bits into one u8 slot per (group, feature).
  - The device emits packed comparison words (V = 64 u8 slots per row); the
    host walks the depth-8 tree on the packed words while unsharding
    (integer numpy, exact) and expands leaf -> value[leaf].  Bitwise exact
    vs the reference (device f32 is_gt == reference compare).
"""

from contextlib import ExitStack

import numpy as np

import concourse.bacc as bacc
import concourse.mybir as mybir
import concourse.tile as tile
from concourse.bass_utils import run_bass_kernel_spmd

AF = mybir.AluOpType
F32 = mybir.dt.float32
U8 = mybir.dt.uint8

N_CORES = 8
P = 128               # SBUF partitions
B_TOTAL = 524288
B_CORE = B_TOTAL // N_CORES      # 65536
S_CORE = B_CORE // P             # 512 rows per partition
F = 32
DEPTH = 8
N_BRANCH = 255
N_LEAF = 256
N_OUT = 8


def _layout_tables(feature, threshold):
    """Plane-major entry layout.

    Entry e = i*32 + f holds the i-th node whose split feature is f ("plane"
    i); unused entries are padded with +inf so their comparison bit is 0.
    Planes come in groups of 8; the fold tree packs each group's bits into
    one u8 word per feature: slot (i//8)*32 + f, bit i%8.
    """
    nodes_by_f = [[] for _ in range(F)]
    for n in range(N_BRANCH):
        nodes_by_f[int(feature[n])].append(n)
    maxc = max(len(v) for v in nodes_by_f)
    # power-of-2 group sizes (<= 8 planes each) covering maxc, e.g. 12 -> [8, 4]
    groups = []
    rem = maxc
    while rem > 0:
        g = 8
        while g > 1 and g // 2 >= rem:
            g //= 2
        groups.append(g)
        rem -= g
    NPL = sum(groups)                # planes
    E = NPL * F                      # table entries
    V = len(groups) * F              # packed u8 slots per row
    plane_of_i = []                  # i -> (group, bit)
    for gi, g in enumerate(groups):
        for j in range(g):
            plane_of_i.append((gi, j))
    th_entries = np.full(E, np.inf, dtype=np.float32)
    slot_lut = np.zeros(N_BRANCH, dtype=np.int64)
    j_lut = np.zeros(N_BRANCH, dtype=np.int64)
    for f, nl in enumerate(nodes_by_f):
        for i, n in enumerate(nl):
            th_entries[i * F + f] = threshold[n]
            gi, j = plane_of_i[i]
            slot_lut[n] = gi * F + f
            j_lut[n] = j
    return groups, NPL, E, V, th_entries, slot_lut, j_lut


def build_nc(feature, threshold, T=2, repeat=1):
    """Single-core Bass program (SPMD: same program on all cores).

    repeat: run the whole pipeline `repeat` times (same output) -- used to
    measure HW kernel time as the wall-clock slope over repeats.
    """
    groups, NPL, E, V, th_entries, _, _ = _layout_tables(feature, threshold)
    S = S_CORE
    assert S % T == 0
    R = S // T
    # +8 row pitch: keeps the row dim unmergeable with the entry dims so no
    # lowered engine-AP dim exceeds the 16-bit ISA num_elem field.
    Ep = E + 8

    nc = bacc.Bacc()
    x = nc.dram_tensor("x", [P * S, F], F32, kind="ExternalInput")
    out = nc.dram_tensor("out", [P, S, E], U8, kind="ExternalOutput")
    xv = x[:].rearrange("(p s) f -> p s f", p=P)

    th_dram = nc.inline_tensor(np.tile(th_entries[None, :], (P, 1)), name="the")

    with ExitStack() as ctx:
        tc = ctx.enter_context(tile.TileContext(nc))
        cpool = ctx.enter_context(tc.tile_pool(name="const", bufs=1))
        pool = ctx.enter_context(tc.tile_pool(name="sb", bufs=1))

        th_t = cpool.tile([P, E], F32, tag="th")
        nc.sync.dma_start(out=th_t[:], in_=th_dram[:])

        xt = pool.tile([P, S, F], F32, tag="x")
        cw = pool.tile([P, R, Ep], U8, tag="cw")

        for rep in range(repeat):
            # one 8MB load covers both tiles of this repeat
            nc.sync.dma_start(out=xt[:], in_=xv[:])
            for t in range(T):
                sl = slice(t * R, (t + 1) * R)

                # cw[r, i*32+f] = x[r, f] > th_entries[i*32+f]
                # (stride-0 broadcast middle dim; inner dim contiguous)
                x_exp = xt[:, sl, :].unsqueeze(2).broadcast_to([P, R, NPL, F])
                th_exp = (th_t[:].rearrange("p (q f) -> p q f", f=F)
                          .unsqueeze(1).broadcast_to([P, R, NPL, F]))
                cw_v = cw[:, :, 0:E].rearrange("p r (q f) -> p r q f", f=F)
                nc.vector.tensor_tensor(out=cw_v, in0=x_exp, in1=th_exp,
                                        op=AF.is_gt)

                # raw 0/1 bytes straight to DRAM; the host reads bits from
                # them directly (DMA is nearly free in this environment,
                # on-device bit-packing is not)
                nc.sync.dma_start(out=out[:][:, sl, :], in_=cw[:, :, 0:E])

    nc.compile()
    return nc


def _check_tree(cond, cond_mask):
    """Verify cond/cond_mask encode the canonical heap-ordered perfect tree."""
    n_nodes = 2 * N_LEAF - 1
    n_branch = N_LEAF - 1
    is_branch = np.zeros(n_nodes, dtype=bool)
    node_conditions = np.zeros((n_nodes, n_nodes), dtype=bool)
    node_conditions_mask = np.zeros((n_nodes, n_nodes), dtype=bool)

    stack = [(0, None)]
    while stack:
        node_id, parent_id = stack.pop()
        if parent_id is not None:
            node_conditions_mask[node_id] = node_conditions_mask[parent_id]
            node_conditions_mask[node_id][parent_id] = True
        if node_id < n_branch:
            left_id, right_id = 2 * node_id + 1, 2 * node_id + 2
            is_branch[node_id] = True
            node_conditions[left_id] = node_conditions[node_id]
            node_conditions[right_id] = node_conditions[node_id]
            node_conditions[right_id][node_id] = True
            stack.append((right_id, node_id))
            stack.append((left_id, node_id))

    leaf_ids = np.nonzero(~is_branch)[0]
    branch_ids = np.nonzero(is_branch)[0]
    c = node_conditions[np.ix_(leaf_ids, branch_ids)]
    m = node_conditions_mask[np.ix_(leaf_ids, branch_ids)]
    return np.array_equal(c, np.asarray(cond)) and np.array_equal(
        m, np.asarray(cond_mask)
    )


_NC_CACHE = {}


def kernel(x, feature, threshold, cond, cond_mask, value):
    x = np.ascontiguousarray(np.asarray(x), dtype=np.float32)
    feature = np.asarray(feature)
    threshold = np.asarray(threshold, dtype=np.float32)
    value = np.ascontiguousarray(np.asarray(value), dtype=np.float32)

    assert x.shape == (B_TOTAL, F), x.shape
    if not _check_tree(cond, cond_mask):
        raise ValueError(
            "cond/cond_mask do not encode the canonical heap-ordered tree; "
            "this kernel bakes that structure."
        )

    key = (feature.tobytes(), threshold.tobytes())
    if key not in _NC_CACHE:
        _NC_CACHE[key] = build_nc(feature, threshold)
    nc = _NC_CACHE[key]

    shards = x.reshape(N_CORES, B_CORE, F)
    in_maps = [{"x": shards[i]} for i in range(N_CORES)]
    res = run_bass_kernel_spmd(nc, in_maps, list(range(N_CORES)))
    return decode_out(
        [np.asarray(r["out"]) for r in res.results], feature, threshold, value
    )


def decode_out(core_outs, feature, threshold, value):
    """Unshard: walk the tree on raw comparison bytes, expand value[leaf]."""
    _, _, E, _, _, _, _ = _layout_tables(feature, threshold)
    # entry index of node n in the plane-major table: e = i*F + f
    nodes_by_f = [[] for _ in range(F)]
    for n in range(N_BRANCH):
        nodes_by_f[int(feature[n])].append(n)
    entry_lut = np.zeros(N_BRANCH, dtype=np.int64)
    for f, nl in enumerate(nodes_by_f):
        for i, n in enumerate(nl):
            entry_lut[n] = i * F + f
    value = np.asarray(value, dtype=np.float32)
    words = np.concatenate(
        [np.asarray(o).reshape(B_CORE, E) for o in core_outs], axis=0
    )                                             # [B, E] u8 of 0/1
    B = words.shape[0]
    rows = np.arange(B)
    n = np.zeros(B, dtype=np.int64)
    for _ in range(DEPTH):
        bits = words[rows, entry_lut[n]]
        n = 2 * n + 1 + bits
    leaf = n - N_BRANCH
    return value[leaf]


if __name__ == "__main__":
    import jax
    import reference

    cpu = jax.devices("cpu")[0]
    with jax.default_device(cpu):
        inputs = {k: np.asarray(v) for k, v in reference.setup_inputs().items()}
        exp = np.asarray(reference.reference(**{
            k: jax.device_put(v, cpu) for k, v in inputs.items()
        }))
    got = kernel(**inputs)
    err = np.abs(got - exp).max()
    print("absmax err:", err)


# revision 7
# speedup vs baseline: 1.6105x; 1.6105x over previous
"""Trainium2 Bass kernel for nn_BaseTree (decision-tree inference).

Problem: x [524288, 32] f32; perfect binary tree depth 8 (255 branch nodes,
256 leaves); out[b] = value[leaf(b)] where leaf(b) is found by descending the
tree: at node n go right iff x[b, feature[n]] > threshold[n].

This environment (axon/PJRT bass execution) runs bass instructions fully
serialized with a ~25-30us fixed overhead per instruction plus a strong
penalty for non-contiguous inner access patterns (contiguous u8 ops stream at
~0.34ns/elem; gather ops cost ~177us each) -- measured via probes.  The
kernel is therefore shaped to minimize instruction count and keep every hot
op's inner dimension contiguous:

  - Pure data parallel across 8 cores (65536 rows each); tree tables are
    baked into the compiled program as immediates.
  - Device, per 256-row-per-partition tile (2 tiles per core, ~9
    instructions each): ONE giant is_gt computes all 255 node comparisons
    per row against a plane-major threshold table (entry e = plane*32 + f
    holds the plane-th node splitting on feature f, padded with +inf).  The
    input AP broadcasts x[r, f] across planes with a stride-0 MIDDLE dim so
    the innermost dim stays contiguous.  Then 3 scalar_tensor_tensor folds
    per 8-plane group ((hi*2^k) + lo on contiguous 128/64/32-entry blocks)
    pack the bits into one u8 slot per (group, feature).
  - The device emits packed comparison words (V = 64 u8 slots per row); the
    host walks the depth-8 tree on the packed words while unsharding
    (integer numpy, exact) and expands leaf -> value[leaf].  Bitwise exact
    vs the reference (device f32 is_gt == reference compare).
"""

from contextlib import ExitStack

import numpy as np

import concourse.bacc as bacc
import concourse.mybir as mybir
import concourse.tile as tile
from concourse.bass_utils import run_bass_kernel_spmd

AF = mybir.AluOpType
F32 = mybir.dt.float32
U8 = mybir.dt.uint8

N_CORES = 8
P = 128               # SBUF partitions
B_TOTAL = 524288
B_CORE = B_TOTAL // N_CORES      # 65536
S_CORE = B_CORE // P             # 512 rows per partition
F = 32
DEPTH = 8
N_BRANCH = 255
N_LEAF = 256
N_OUT = 8


def _layout_tables(feature, threshold):
    """Plane-major entry layout.

    Entry e = i*32 + f holds the i-th node whose split feature is f ("plane"
    i); unused entries are padded with +inf so their comparison bit is 0.
    Planes come in groups of 8; the fold tree packs each group's bits into
    one u8 word per feature: slot (i//8)*32 + f, bit i%8.
    """
    nodes_by_f = [[] for _ in range(F)]
    for n in range(N_BRANCH):
        nodes_by_f[int(feature[n])].append(n)
    maxc = max(len(v) for v in nodes_by_f)
    # power-of-2 group sizes (<= 8 planes each) covering maxc, e.g. 12 -> [8, 4]
    groups = []
    rem = maxc
    while rem > 0:
        g = 8
        while g > 1 and g // 2 >= rem:
            g //= 2
        groups.append(g)
        rem -= g
    NPL = sum(groups)                # planes
    E = NPL * F                      # table entries
    V = len(groups) * F              # packed u8 slots per row
    plane_of_i = []                  # i -> (group, bit)
    for gi, g in enumerate(groups):
        for j in range(g):
            plane_of_i.append((gi, j))
    th_entries = np.full(E, np.inf, dtype=np.float32)
    slot_lut = np.zeros(N_BRANCH, dtype=np.int64)
    j_lut = np.zeros(N_BRANCH, dtype=np.int64)
    for f, nl in enumerate(nodes_by_f):
        for i, n in enumerate(nl):
            th_entries[i * F + f] = threshold[n]
            gi, j = plane_of_i[i]
            slot_lut[n] = gi * F + f
            j_lut[n] = j
    return groups, NPL, E, V, th_entries, slot_lut, j_lut


def build_nc(feature, threshold, T=2, repeat=1):
    """Single-core Bass program (SPMD: same program on all cores).

    repeat: run the whole pipeline `repeat` times (same output) -- used to
    measure HW kernel time as the wall-clock slope over repeats.
    """
    groups, NPL, E, V, th_entries, _, _ = _layout_tables(feature, threshold)
    S = S_CORE
    assert S % T == 0
    R = S // T
    # +8 row pitch: keeps the row dim unmergeable with the entry dims so no
    # lowered engine-AP dim exceeds the 16-bit ISA num_elem field.
    Ep = E + 8

    nc = bacc.Bacc()
    x = nc.dram_tensor("x", [P * S, F], F32, kind="ExternalInput")
    out = nc.dram_tensor("out", [P, S, V], U8, kind="ExternalOutput")
    xv = x[:].rearrange("(p s) f -> p s f", p=P)

    th_dram = nc.inline_tensor(np.tile(th_entries[None, :], (P, 1)), name="the")

    with ExitStack() as ctx:
        tc = ctx.enter_context(tile.TileContext(nc))
        cpool = ctx.enter_context(tc.tile_pool(name="const", bufs=1))
        pool = ctx.enter_context(tc.tile_pool(name="sb", bufs=1))

        th_t = cpool.tile([P, E], F32, tag="th")
        nc.sync.dma_start(out=th_t[:], in_=th_dram[:])

        xt = pool.tile([P, S, F], F32, tag="x")
        cw = pool.tile([P, R, Ep], U8, tag="cw")
        wl = pool.tile([P, S, V], U8, tag="wl")

        for rep in range(repeat):
            # one 8MB load covers both tiles of this repeat
            nc.sync.dma_start(out=xt[:], in_=xv[:])
            for t in range(T):
                sl = slice(t * R, (t + 1) * R)

                # cw[r, i*32+f] = x[r, f] > th_entries[i*32+f]
                # (stride-0 broadcast middle dim; inner dim contiguous)
                x_exp = xt[:, sl, :].unsqueeze(2).broadcast_to([P, R, NPL, F])
                th_exp = (th_t[:].rearrange("p (q f) -> p q f", f=F)
                          .unsqueeze(1).broadcast_to([P, R, NPL, F]))
                cw_v = cw[:, :, 0:E].rearrange("p r (q f) -> p r q f", f=F)
                nc.vector.tensor_tensor(out=cw_v, in0=x_exp, in1=th_exp,
                                        op=AF.is_gt)

                # pack each power-of-2 plane group: log2(g) folds of
                # (hi_half * 2^half) + lo_half over contiguous blocks;
                # wl slot value = sum_j bit_j * 2^j.
                base = 0
                for gi, gsz in enumerate(groups):

                    def blk(lo, hi, base=base):
                        return cw[:, :, base + lo * F: base + hi * F]

                    half = gsz // 2
                    while half >= 1:
                        dst = (wl[:, sl, gi * F:(gi + 1) * F] if half == 1
                               else blk(0, half))
                        nc.vector.scalar_tensor_tensor(
                            out=dst, in0=blk(half, 2 * half), scalar=1 << half,
                            in1=blk(0, half), op0=AF.mult, op1=AF.add)
                        half //= 2
                    base += gsz * F

            # single out-DMA covers both tiles
            nc.sync.dma_start(out=out[:], in_=wl[:])

    nc.compile()
    return nc


def _check_tree(cond, cond_mask):
    """Verify cond/cond_mask encode the canonical heap-ordered perfect tree."""
    n_nodes = 2 * N_LEAF - 1
    n_branch = N_LEAF - 1
    is_branch = np.zeros(n_nodes, dtype=bool)
    node_conditions = np.zeros((n_nodes, n_nodes), dtype=bool)
    node_conditions_mask = np.zeros((n_nodes, n_nodes), dtype=bool)

    stack = [(0, None)]
    while stack:
        node_id, parent_id = stack.pop()
        if parent_id is not None:
            node_conditions_mask[node_id] = node_conditions_mask[parent_id]
            node_conditions_mask[node_id][parent_id] = True
        if node_id < n_branch:
            left_id, right_id = 2 * node_id + 1, 2 * node_id + 2
            is_branch[node_id] = True
            node_conditions[left_id] = node_conditions[node_id]
            node_conditions[right_id] = node_conditions[node_id]
            node_conditions[right_id][node_id] = True
            stack.append((right_id, node_id))
            stack.append((left_id, node_id))

    leaf_ids = np.nonzero(~is_branch)[0]
    branch_ids = np.nonzero(is_branch)[0]
    c = node_conditions[np.ix_(leaf_ids, branch_ids)]
    m = node_conditions_mask[np.ix_(leaf_ids, branch_ids)]
    return np.array_equal(c, np.asarray(cond)) and np.array_equal(
        m, np.asarray(cond_mask)
    )


_NC_CACHE = {}


def kernel(x, feature, threshold, cond, cond_mask, value):
    x = np.ascontiguousarray(np.asarray(x), dtype=np.float32)
    feature = np.asarray(feature)
    threshold = np.asarray(threshold, dtype=np.float32)
    value = np.ascontiguousarray(np.asarray(value), dtype=np.float32)

    assert x.shape == (B_TOTAL, F), x.shape
    if not _check_tree(cond, cond_mask):
        raise ValueError(
            "cond/cond_mask do not encode the canonical heap-ordered tree; "
            "this kernel bakes that structure."
        )

    key = (feature.tobytes(), threshold.tobytes())
    if key not in _NC_CACHE:
        _NC_CACHE[key] = build_nc(feature, threshold)
    nc = _NC_CACHE[key]

    shards = x.reshape(N_CORES, B_CORE, F)
    in_maps = [{"x": shards[i]} for i in range(N_CORES)]
    res = run_bass_kernel_spmd(nc, in_maps, list(range(N_CORES)))
    return decode_out(
        [np.asarray(r["out"]) for r in res.results], feature, threshold, value
    )


def decode_out(core_outs, feature, threshold, value):
    """Unshard: walk the tree on packed comparison words, expand value[leaf]."""
    _, _, _, V, _, slot_lut, j_lut = _layout_tables(feature, threshold)
    value = np.asarray(value, dtype=np.float32)
    words = np.concatenate(
        [np.asarray(o).reshape(B_CORE, V) for o in core_outs], axis=0
    )                                             # [B, V] u8
    B = words.shape[0]
    rows = np.arange(B)
    n = np.zeros(B, dtype=np.int64)
    for _ in range(DEPTH):
        bits = (words[rows, slot_lut[n]] >> j_lut[n]) & 1
        n = 2 * n + 1 + bits
    leaf = n - N_BRANCH
    return value[leaf]


if __name__ == "__main__":
    import jax
    import reference

    cpu = jax.devices("cpu")[0]
    with jax.default_device(cpu):
        inputs = {k: np.asarray(v) for k, v in reference.setup_inputs().items()}
        exp = np.asarray(reference.reference(**{
            k: jax.device_put(v, cpu) for k, v in inputs.items()
        }))
    got = kernel(**inputs)
    err = np.abs(got - exp).max()
    print("absmax err:", err)


# revision 8
# speedup vs baseline: 1.9550x; 1.2139x over previous
"""Trainium2 Bass kernel for nn_BaseTree (decision-tree inference).

Problem: x [524288, 32] f32; perfect binary tree depth 8 (255 branch nodes,
256 leaves); out[b] = value[leaf(b)] where leaf(b) is found by descending the
tree: at node n go right iff x[b, feature[n]] > threshold[n].

This environment (axon/PJRT bass execution) runs bass instructions fully
serialized with a ~25-30us fixed overhead per instruction plus a strong
penalty for non-contiguous inner access patterns (contiguous u8 ops stream at
~0.34ns/elem; gather ops cost ~177us each) -- measured via probes.  The
kernel is therefore shaped to minimize instruction count and keep every hot
op's inner dimension contiguous:

  - Pure data parallel across 8 cores (65536 rows each); tree tables are
    baked into the compiled program as immediates.
  - Device, per 256-row-per-partition tile (2 tiles per core, ~9
    instructions each): ONE giant is_gt computes all 255 node comparisons
    per row against a plane-major threshold table (entry e = plane*32 + f
    holds the plane-th node splitting on feature f, padded with +inf).  The
    input AP broadcasts x[r, f] across planes with a stride-0 MIDDLE dim so
    the innermost dim stays contiguous.  Then 3 scalar_tensor_tensor folds
    per 8-plane group ((hi*2^k) + lo on contiguous 128/64/32-entry blocks)
    pack the bits into one u8 slot per (group, feature).
  - The device emits packed comparison words (V = 64 u8 slots per row); the
    host walks the depth-8 tree on the packed words while unsharding
    (integer numpy, exact) and expands leaf -> value[leaf].  Bitwise exact
    vs the reference (device f32 is_gt == reference compare).
"""

from contextlib import ExitStack

import numpy as np

import concourse.bacc as bacc
import concourse.mybir as mybir
import concourse.tile as tile
from concourse.bass_utils import run_bass_kernel_spmd

AF = mybir.AluOpType
F32 = mybir.dt.float32
U8 = mybir.dt.uint8

N_CORES = 8
P = 128               # SBUF partitions
B_TOTAL = 524288
B_CORE = B_TOTAL // N_CORES      # 65536
S_CORE = B_CORE // P             # 512 rows per partition
F = 32
DEPTH = 8
N_BRANCH = 255
N_LEAF = 256
N_OUT = 8


def _layout_tables(feature, threshold):
    """Plane-major entry layout.

    Entry e = i*32 + f holds the i-th node whose split feature is f ("plane"
    i); unused entries are padded with +inf so their comparison bit is 0.
    Planes come in groups of 8; the fold tree packs each group's bits into
    one u8 word per feature: slot (i//8)*32 + f, bit i%8.
    """
    nodes_by_f = [[] for _ in range(F)]
    for n in range(N_BRANCH):
        nodes_by_f[int(feature[n])].append(n)
    maxc = max(len(v) for v in nodes_by_f)
    # power-of-2 group sizes (<= 8 planes each) covering maxc, e.g. 12 -> [8, 4]
    groups = []
    rem = maxc
    while rem > 0:
        g = 8
        while g > 1 and g // 2 >= rem:
            g //= 2
        groups.append(g)
        rem -= g
    NPL = sum(groups)                # planes
    E = NPL * F                      # table entries
    V = len(groups) * F              # packed u8 slots per row
    plane_of_i = []                  # i -> (group, bit)
    for gi, g in enumerate(groups):
        for j in range(g):
            plane_of_i.append((gi, j))
    th_entries = np.full(E, np.inf, dtype=np.float32)
    slot_lut = np.zeros(N_BRANCH, dtype=np.int64)
    j_lut = np.zeros(N_BRANCH, dtype=np.int64)
    for f, nl in enumerate(nodes_by_f):
        for i, n in enumerate(nl):
            th_entries[i * F + f] = threshold[n]
            gi, j = plane_of_i[i]
            slot_lut[n] = gi * F + f
            j_lut[n] = j
    return groups, NPL, E, V, th_entries, slot_lut, j_lut


def build_nc(feature, threshold, T=2, repeat=1):
    """Single-core Bass program (SPMD: same program on all cores).

    repeat: run the whole pipeline `repeat` times (same output) -- used to
    measure HW kernel time as the wall-clock slope over repeats.
    """
    groups, NPL, E, V, th_entries, _, _ = _layout_tables(feature, threshold)
    S = S_CORE
    assert S % T == 0
    R = S // T
    # +8 row pitch: keeps the row dim unmergeable with the entry dims so no
    # lowered engine-AP dim exceeds the 16-bit ISA num_elem field.
    Ep = E + 8

    nc = bacc.Bacc()
    x = nc.dram_tensor("x", [P * S, F], F32, kind="ExternalInput")
    out = nc.dram_tensor("out", [P, S, V], U8, kind="ExternalOutput")
    xv = x[:].rearrange("(p s) f -> p s f", p=P)

    th_dram = nc.inline_tensor(np.tile(th_entries[None, :], (P, 1)), name="the")

    with ExitStack() as ctx:
        tc = ctx.enter_context(tile.TileContext(nc))
        cpool = ctx.enter_context(tc.tile_pool(name="const", bufs=1))
        pool = ctx.enter_context(tc.tile_pool(name="sb", bufs=1))

        th_t = cpool.tile([P, E], F32, tag="th")
        nc.sync.dma_start(out=th_t[:], in_=th_dram[:])

        xt = pool.tile([P, R, F], F32, tag="x")
        cw = pool.tile([P, R, Ep], U8, tag="cw")
        wl = pool.tile([P, R, V], U8, tag="wl")

        for rep_t in range(T * repeat):
            t = rep_t % T
            sl = slice(t * R, (t + 1) * R)
            nc.sync.dma_start(out=xt[:], in_=xv[:, sl, :])

            # cw[r, i*32+f] = x[r, f] > th_entries[i*32+f]
            # (stride-0 broadcast on the middle dim; inner dim contiguous)
            x_exp = xt[:].unsqueeze(2).broadcast_to([P, R, NPL, F])
            th_exp = (th_t[:].rearrange("p (q f) -> p q f", f=F)
                      .unsqueeze(1).broadcast_to([P, R, NPL, F]))
            cw_v = cw[:, :, 0:E].rearrange("p r (q f) -> p r q f", f=F)
            nc.vector.tensor_tensor(out=cw_v, in0=x_exp, in1=th_exp,
                                    op=AF.is_gt)

            # pack each power-of-2 plane group: log2(g) folds of
            # (hi_half * 2^half) + lo_half over contiguous blocks;
            # wl slot value = sum_j bit_j * 2^j.
            base = 0
            for gi, gsz in enumerate(groups):

                def blk(lo, hi, base=base):
                    return cw[:, :, base + lo * F: base + hi * F]

                half = gsz // 2
                while half >= 1:
                    dst = (wl[:, :, gi * F:(gi + 1) * F] if half == 1
                           else blk(0, half))
                    nc.vector.scalar_tensor_tensor(
                        out=dst, in0=blk(half, 2 * half), scalar=1 << half,
                        in1=blk(0, half), op0=AF.mult, op1=AF.add)
                    half //= 2
                base += gsz * F

            nc.sync.dma_start(out=out[:][:, sl, :], in_=wl[:])

    nc.compile()
    return nc


def _check_tree(cond, cond_mask):
    """Verify cond/cond_mask encode the canonical heap-ordered perfect tree."""
    n_nodes = 2 * N_LEAF - 1
    n_branch = N_LEAF - 1
    is_branch = np.zeros(n_nodes, dtype=bool)
    node_conditions = np.zeros((n_nodes, n_nodes), dtype=bool)
    node_conditions_mask = np.zeros((n_nodes, n_nodes), dtype=bool)

    stack = [(0, None)]
    while stack:
        node_id, parent_id = stack.pop()
        if parent_id is not None:
            node_conditions_mask[node_id] = node_conditions_mask[parent_id]
            node_conditions_mask[node_id][parent_id] = True
        if node_id < n_branch:
            left_id, right_id = 2 * node_id + 1, 2 * node_id + 2
            is_branch[node_id] = True
            node_conditions[left_id] = node_conditions[node_id]
            node_conditions[right_id] = node_conditions[node_id]
            node_conditions[right_id][node_id] = True
            stack.append((right_id, node_id))
            stack.append((left_id, node_id))

    leaf_ids = np.nonzero(~is_branch)[0]
    branch_ids = np.nonzero(is_branch)[0]
    c = node_conditions[np.ix_(leaf_ids, branch_ids)]
    m = node_conditions_mask[np.ix_(leaf_ids, branch_ids)]
    return np.array_equal(c, np.asarray(cond)) and np.array_equal(
        m, np.asarray(cond_mask)
    )


_NC_CACHE = {}


def kernel(x, feature, threshold, cond, cond_mask, value):
    x = np.ascontiguousarray(np.asarray(x), dtype=np.float32)
    feature = np.asarray(feature)
    threshold = np.asarray(threshold, dtype=np.float32)
    value = np.ascontiguousarray(np.asarray(value), dtype=np.float32)

    assert x.shape == (B_TOTAL, F), x.shape
    if not _check_tree(cond, cond_mask):
        raise ValueError(
            "cond/cond_mask do not encode the canonical heap-ordered tree; "
            "this kernel bakes that structure."
        )

    key = (feature.tobytes(), threshold.tobytes())
    if key not in _NC_CACHE:
        _NC_CACHE[key] = build_nc(feature, threshold)
    nc = _NC_CACHE[key]

    shards = x.reshape(N_CORES, B_CORE, F)
    in_maps = [{"x": shards[i]} for i in range(N_CORES)]
    res = run_bass_kernel_spmd(nc, in_maps, list(range(N_CORES)))
    return decode_out(
        [np.asarray(r["out"]) for r in res.results], feature, threshold, value
    )


def decode_out(core_outs, feature, threshold, value):
    """Unshard: walk the tree on packed comparison words, expand value[leaf]."""
    _, _, _, V, _, slot_lut, j_lut = _layout_tables(feature, threshold)
    value = np.asarray(value, dtype=np.float32)
    words = np.concatenate(
        [np.asarray(o).reshape(B_CORE, V) for o in core_outs], axis=0
    )                                             # [B, V] u8
    B = words.shape[0]
    rows = np.arange(B)
    n = np.zeros(B, dtype=np.int64)
    for _ in range(DEPTH):
        bits = (words[rows, slot_lut[n]] >> j_lut[n]) & 1
        n = 2 * n + 1 + bits
    leaf = n - N_BRANCH
    return value[leaf]


if __name__ == "__main__":
    import jax
    import reference

    cpu = jax.devices("cpu")[0]
    with jax.default_device(cpu):
        inputs = {k: np.asarray(v) for k, v in reference.setup_inputs().items()}
        exp = np.asarray(reference.reference(**{
            k: jax.device_put(v, cpu) for k, v in inputs.items()
        }))
    got = kernel(**inputs)
    err = np.abs(got - exp).max()
    print("absmax err:", err)
